# Initial kernel scaffold
#
"""Trainium2 Bass kernel for batched per-sample expert matmul (MoE routing).

Computes y[n, i] = relu(b[idxs[n], i] + sum_o w[idxs[n], i, o] * x[n, o])
for x (8192, 256), idxs (8192,), w (64, 256, 256), b (64, 256).

Strategy
--------
Host side (numpy, cheap):
  * Stable-sort all 8192 samples by expert id, shard the *sorted* batch
    contiguously across the 8 cores (1024 samples each). Each core's
    samples then span only a handful of contiguous experts, so the
    per-core weight traffic is ~3 MB instead of the full 16 MB table.
  * Cut each core's samples into segments of <= 128 samples, one expert
    per segment, padded so every core runs the same NSEG segments
    (SPMD: one program, per-core data).
  * Pre-gather, per segment: the expert's weight matrix laid out for the
    PE (contraction dim on partitions) and its bias row.  Pre-transpose
    the sorted x so the contraction dim is on partitions.

Device side (one static Tile program, identical on all 8 cores):
  for each segment s:
      psum[m, i]  = 1 * b[i]                       (K=1 bias matmul)
      psum[m, i] += sum_p xT0[p, m] * w0[p, i]     (K-chunk 0)
      psum[m, i] += sum_p xT1[p, m] * w1[p, i]     (K-chunk 1)
      y[m, i]     = relu(psum[m, i])               (ACT, PSUM -> SBUF)

  This walrus build allows only ONE semaphore wait on lowered matmul
  (LDWEIGHTS+MATMUL) and HWDGE direct-DMA instructions, so the program
  is shaped so nothing ever needs two: <= 8 DMAs total (no DMA-lane
  recycling), each weight batch in its own SBUF slot (no WAW waits),
  the bias matmul opens each PSUM group (operands resident; carries
  only the PSUM-release wait), K-chunk matmuls carry only the
  weight-DMA wait (x residency absorbed by a prologue dummy matmul),
  and ones/bias share one DMA.

Host side: scatter segment rows back to the original sample order.
Inputs with pathological expert skew can exceed the per-pass segment
budget; those run the same program over multiple passes.
"""

import os

import numpy as np

import concourse.bacc as bacc
import concourse.bass as bass
import concourse.mybir as mybir
import concourse.tile as tile
from concourse.bass_utils import run_bass_kernel_spmd

N_CORES = 8
P = 128          # SBUF/PSUM partitions
F = 256          # feature dim (in_features == out_features == 256)
SEG = 128        # samples per segment (== max PSUM partition dim)
OGS = 4          # segments per output-DMA batch
MAX_NSEG = 32    # per-pass segment budget (SBUF residency bound)


def _batches(n, sizes, rest):
    """Split range(n) into batches: explicit `sizes` first, then `rest`-sized."""
    out = []
    lo = 0
    i = 0
    while lo < n:
        sz = sizes[i] if i < len(sizes) else rest
        i += 1
        hi = min(n, lo + sz)
        out.append((lo, hi))
        lo = hi
    return out


def _stream_batches(n):
    """Batch plans for the weight and x streams over n segments.

    Small head batches (compute starts early), large middle ones (amortize
    the per-DMA fixed cost), a tiny tail batch (minimal compute left after
    the stream ends). x-batch starts are staggered against w-batch starts
    so no K-chunk matmul needs both an x- and a w-DMA wait.
    """
    if n <= 4:
        wbat = _batches(n, [1], 2)
    else:
        mid = n - 8
        wsz = [1, 2] + ([4] * (mid // 4)) + ([mid % 4] if mid % 4 else []) + [4, 1]
        wbat = _batches(n, wsz, 4)
    wstarts = {lo for lo, _ in wbat}
    starts = [0]
    for lo, _ in _batches(n, [2, 5], 6)[1:]:
        while lo in wstarts or lo <= starts[-1]:
            lo += 1
        if lo < n:
            starts.append(lo)
    xbat = [
        (starts[i], starts[i + 1] if i + 1 < len(starts) else n)
        for i in range(len(starts))
    ]
    return wbat, xbat
MM_DT = (
    mybir.dt.float32
    if os.environ.get("KBENCH_MM_DT", "float32r") == "float32"
    else mybir.dt.float32r
)  # matmul operand dtype; float32r streams 4x faster at moving dim >= 256

# Set by the last kernel() call when KBENCH_TRACE=1 (used by test.py only).
LAST_EXEC_TIME_NS = None
LAST_TRACE = None


def _build_schedule(idxs: np.ndarray):
    """Sort samples by expert, shard, and cut per-core single-expert segments."""
    B = idxs.shape[0]
    S = B // N_CORES
    order = np.argsort(idxs, kind="stable")
    sidx = idxs[order]
    per_core = []
    for c in range(N_CORES):
        e = sidx[c * S:(c + 1) * S]
        # run-length encode the (sorted) expert ids of this core's chunk
        segs = []  # (expert, local_start, count), count <= SEG
        i = 0
        while i < S:
            j = i
            while j < S and e[j] == e[i]:
                j += 1
            k = i
            while k < j:
                cnt = min(SEG, j - k)
                segs.append((int(e[i]), k, cnt))
                k += cnt
            i = j
        per_core.append(segs)
    return order, per_core


def _build_program(nseg: int):
    nc = bacc.Bacc(
        "TRN2", target_bir_lowering=False, debug=False, num_devices=N_CORES
    )
    npad = nseg * SEG
    xt_d = nc.dram_tensor("xt", [P, 2, npad], MM_DT, kind="ExternalInput").ap()
    w_d = nc.dram_tensor("wseg", [P, nseg, 2 * F], MM_DT, kind="ExternalInput").ap()
    b_d = nc.dram_tensor(
        "bconst", [1, P + nseg * F], MM_DT, kind="ExternalInput"
    ).ap()
    y_d = nc.dram_tensor(
        "y", [P, nseg, F], mybir.dt.float32, kind="ExternalOutput"
    ).ap()

    f32 = mybir.dt.float32
    relu = mybir.ActivationFunctionType.Relu

    wbat, xbat = _stream_batches(nseg)
    obat = _batches(nseg, [], OGS)

    with tile.TileContext(nc) as tc:
        with (
            tc.tile_pool(name="const", bufs=1) as const,
            tc.tile_pool(name="w", bufs=1) as wpool,
            tc.tile_pool(name="yout", bufs=1) as ypool,
            tc.tile_pool(name="ps", bufs=3, space="PSUM") as pspool,
            tc.tile_pool(name="scr", bufs=1, space="PSUM") as scrpool,
        ):
            # bconst rides the otherwise-idle HWDGE ring; the x/w streams go
            # through the single SWDGE queue (gpsimd): FIFO delivery in issue
            # order at full line rate, one completion semaphore per batch ->
            # a just-in-time pipeline.  (Independent HWDGE queues share SDMA
            # bandwidth round-robin, which delays the earliest transfer.)
            bc = const.tile([1, P + nseg * F], MM_DT, tag="bconst")
            nc.sync.dma_start(bc[:], b_d[:])

            xts = {}

            def load_x_batch(b):
                lo, hi = xbat[b]
                t = const.tile([P, 2 * (hi - lo) * SEG], MM_DT, tag=f"xt{b}")
                xts[b] = t
                nc.gpsimd.dma_start(
                    t[:].rearrange("p (c n) -> p c n", c=2),
                    xt_d[:, :, lo * SEG:hi * SEG],
                )

            wts = {}

            def load_w_batch(g):
                lo, hi = wbat[g]
                t = wpool.tile([P, (hi - lo) * 2 * F], MM_DT, tag=f"w{g}")
                wts[g] = t
                nc.gpsimd.dma_start(
                    t[:], w_d[:, lo:hi, :].rearrange("p g f -> p (g f)")
                )

            ones = bc[:, 0:P]

            seg2x = {}
            for b, (lo, hi) in enumerate(xbat):
                for s in range(lo, hi):
                    seg2x[s] = b
            seg2w = {}
            for g, (lo, hi) in enumerate(wbat):
                for s in range(lo, hi):
                    seg2w[s] = g

            def xchunk(s, c):
                b = seg2x[s]
                lo, hi = xbat[b]
                base = (c * (hi - lo) + (s - lo)) * SEG
                return xts[b][:, base:base + SEG]

            def wchunk(s, c):
                g = seg2w[s]
                lo, hi = wbat[g]
                base = (2 * (s - lo) + c) * F
                return wts[g][:, base:base + F]

            # need-order emission on the SWDGE queue
            nxt_x = 0
            for g in range(len(wbat)):
                while nxt_x < len(xbat) and xbat[nxt_x][0] <= wbat[g][0]:
                    load_x_batch(nxt_x)
                    nxt_x += 1
                load_w_batch(g)
            while nxt_x < len(xbat):
                load_x_batch(nxt_x)
                nxt_x += 1

            scr = scrpool.tile([2, 2], f32)
            yt = None
            # Process segments in pairs sharing one full PSUM bank: a single
            # N=512 bias matmul covers both, and one ACT relu drains both.
            pairs = _batches(nseg, [], 2)
            for plo, phi in pairs:
                pw = (phi - plo) * F
                if plo % OGS == 0:
                    yt = ypool.tile([P, OGS * F], f32, tag=f"y{plo // OGS}")
                if plo == 0:
                    # Absorb x batch 0's DMA wait into the PE's clock so
                    # segment 0's K-chunk matmul only needs the w-DMA wait.
                    xb = xts[0]
                    nc.tensor.matmul(
                        scr[:], xb[:, 0:2], xb[:, 0:2], start=True, stop=True
                    )
                ps = pspool.tile([P, 2 * F], f32)
                nc.tensor.matmul(
                    ps[:, 0:pw],
                    ones,
                    bc[:, P + plo * F:P + phi * F],
                    start=True,
                    stop=False,
                )
                for s in range(plo, phi):
                    o = (s - plo) * F
                    nc.tensor.matmul(
                        ps[:, o:o + F], xchunk(s, 0), wchunk(s, 0),
                        start=False, stop=False,
                    )
                    nc.tensor.matmul(
                        ps[:, o:o + F], xchunk(s, 1), wchunk(s, 1),
                        start=False, stop=(s == phi - 1),
                    )
                j = plo % OGS
                # relu on DVE: keeps ACT (and its 1.3us table-load preamble)
                # out of the kernel entirely.
                nc.vector.tensor_scalar_max(
                    yt[:, j * F:j * F + pw], ps[:, 0:pw], 0.0
                )
                if phi % OGS == 0 or phi == nseg:
                    lo = (plo // OGS) * OGS
                    nc.sync.dma_start(
                        y_d[:, lo:phi, :].rearrange("p g f -> p (g f)"),
                        yt[:, 0:(phi - lo) * F],
                    )
    nc.compile()
    return nc


def kernel(x: np.ndarray, idxs: np.ndarray, w: np.ndarray, b: np.ndarray) -> np.ndarray:
    global LAST_EXEC_TIME_NS, LAST_TRACE
    x = np.ascontiguousarray(x, dtype=np.float32)
    w = np.ascontiguousarray(w, dtype=np.float32)
    b = np.ascontiguousarray(b, dtype=np.float32)
    idxs_np = np.asarray(idxs).astype(np.int64)

    B = x.shape[0]
    S = B // N_CORES
    order, per_core = _build_schedule(idxs_np)

    # Split each core's segment list into passes of <= MAX_NSEG segments.
    npass = max(1, (max(len(s) for s in per_core) + MAX_NSEG - 1) // MAX_NSEG)
    if npass == 1:
        nseg = max(2, max(len(s) for s in per_core))
    else:
        nseg = MAX_NSEG
    npad = nseg * SEG

    # Per-expert weight blocks in PE layout:
    # wprep[e, p, c*F + i] = w[e, i, c*P + p]  (c = contraction chunk 0/1)
    wprep = np.ascontiguousarray(
        w.transpose(0, 2, 1)           # (e, o, i)
        .reshape(64, 2, P, F)          # (e, c, p, i)
        .transpose(0, 2, 1, 3)         # (e, p, c, i)
        .reshape(64, P, 2 * F)
    )

    nc = _build_program(nseg)
    trace = bool(os.environ.get("KBENCH_TRACE"))

    y = np.empty((B, F), dtype=np.float32)
    for pi in range(npass):
        in_maps = []
        for c in range(N_CORES):
            sel = order[c * S:(c + 1) * S]
            segs = per_core[c][pi * MAX_NSEG:(pi + 1) * MAX_NSEG]
            xpad = np.zeros((npad, F), dtype=np.float32)
            eids = np.zeros(nseg, dtype=np.int64)
            for s, (e, k0, cnt) in enumerate(segs):
                xpad[s * SEG:s * SEG + cnt] = x[sel[k0:k0 + cnt]]
                eids[s] = e
            # xt[p, c, n] = xpad[n, c*P + p]
            xt = np.ascontiguousarray(
                xpad.T.reshape(2, P, npad).transpose(1, 0, 2)
            )
            wseg = np.ascontiguousarray(
                wprep[eids].transpose(1, 0, 2)
            )  # (P, nseg, 2F)
            bconst = np.concatenate(
                [np.ones(P, dtype=np.float32), b[eids].reshape(nseg * F)]
            ).reshape(1, P + nseg * F)
            in_maps.append({"xt": xt, "wseg": wseg, "bconst": bconst})

        res = run_bass_kernel_spmd(
            nc, in_maps, core_ids=list(range(N_CORES)), trace=trace
        )
        LAST_EXEC_TIME_NS = res.exec_time_ns
        LAST_TRACE = res.instructions_and_trace

        for c in range(N_CORES):
            sel = order[c * S:(c + 1) * S]
            segs = per_core[c][pi * MAX_NSEG:(pi + 1) * MAX_NSEG]
            ypad = res.results[c]["y"].transpose(1, 0, 2).reshape(npad, F)
            for s, (e, k0, cnt) in enumerate(segs):
                y[sel[k0:k0 + cnt]] = ypad[s * SEG:s * SEG + cnt]
    return y



# revision 54
# speedup vs baseline: 1.4208x; 1.4208x over previous
"""Trainium2 Bass kernel for batched per-sample expert matmul (MoE routing).

Computes y[n, i] = relu(b[idxs[n], i] + sum_o w[idxs[n], i, o] * x[n, o])
for x (8192, 256), idxs (8192,), w (64, 256, 256), b (64, 256).

Strategy (v2: expert-aligned shard, weight-stationary dataflow)
---------------------------------------------------------------
Host side (numpy, cheap):
  * Cut the batch into per-expert "units" of <= PAD samples (for the
    nominal input every expert has ~128 +- 15 samples, so unit == expert
    and PAD = 160).  Deal 8 units to each of the 8 cores; every unit has
    the same padded cost, so the load is perfectly balanced.
  * Per core, pre-transpose the unit samples so the contraction dim is
    on partitions (xt[p, c, j] = x_j[c*128 + p]) and slice the weight
    table into PE-stationary chunks (wstat[p, u, oc, ic, m] =
    w[e_u, ic*128+m, oc*128+p]).  All streams are fp16: precision is
    ample (values are O(1), accumulation stays fp32 in PSUM) and DMA
    bytes halve vs fp32.

Device side (one static Tile program, identical on all 8 cores —
per-core behaviour lives entirely in the DMA'd data):
  for each unit u, each output-feature chunk ic (128 features):
      psum[i, j]  = sum_p wstat[p,u,0,ic,i] * xt[p,0,uPAD+j]   (K-chunk 0)
      psum[i, j] += sum_p wstat[p,u,1,ic,i] * xt[p,1,uPAD+j]   (K-chunk 1)
      y[i, j] = relu(psum[i, j] + bias[i])     (drain, alternating DVE/ACT)

  With features on PSUM partitions the bias is a per-partition scalar,
  so it rides the drain op for free — no bias matmuls at all — and the
  weights are the stationary operand, so each expert's weights cross
  HBM exactly once (1 MB/core, a static slice of the table).

  DMA: weights stream on the SWDGE queue (3 batches), x + bias + y on
  the HWDGE ring (descriptor generation runs on different engines, so
  the two streams' issue costs overlap; the SDMA engines are shared and
  bandwidth-bound either way).  Two dummy matmuls absorb the w/x batch-0
  semaphore waits so no real matmul needs two waits; later batch
  boundaries are staggered so each first-consumer carries at most one.

Host side: scatter unit rows back to the original sample order.
"""

import os

import numpy as np

import concourse.bacc as bacc
import concourse.bass as bass
import concourse.mybir as mybir
import concourse.tile as tile
from concourse.bass_utils import run_bass_kernel_spmd

N_CORES = 8
P = 128          # SBUF/PSUM partitions
F = 256          # feature dim (in_features == out_features == 256)
NE = 8           # units (experts) per core in the nominal case

MM_DT = mybir.dt.float16
NP_DT = np.float16
Y_DT = mybir.dt.float16

# Set by the last kernel() call when KBENCH_TRACE=1 (used by test.py only).
LAST_EXEC_TIME_NS = None
LAST_TRACE = None


def _build_units(idxs: np.ndarray, pad_cap: int = 512):
    """Cut the batch into single-expert units of <= pad_cap samples, sort by
    size, and deal them so slot j holds similar-sized units on every core.
    Returns (order, slot_pads, NE_eff, per_core): per_core[c][j] is
    (expert, start_in_order, count) and slot_pads[j] >= count for all cores."""
    order = np.argsort(idxs, kind="stable")
    sidx = idxs[order]
    # run-length encode the sorted expert ids
    bounds = np.flatnonzero(np.r_[True, sidx[1:] != sidx[:-1], True])
    units = []
    for lo, hi in zip(bounds[:-1], bounds[1:]):
        e = int(sidx[lo])
        k = int(lo)
        while k < hi:
            cnt = min(pad_cap, int(hi) - k)
            units.append((e, k, cnt))
            k += cnt
    units.sort(key=lambda u: -u[2])
    n_units = len(units)
    ne = (n_units + N_CORES - 1) // N_CORES
    units += [(0, 0, 0)] * (ne * N_CORES - n_units)
    per_core = [
        [units[j * N_CORES + c] for j in range(ne)] for c in range(N_CORES)
    ]
    slot_pads = [
        max(16, ((max(units[j * N_CORES + c][2] for c in range(N_CORES)) + 15)
                 // 16) * 16)
        for j in range(ne)
    ]
    return order, slot_pads, ne, per_core


def _batches(n, sizes, rest):
    out, lo, i = [], 0, 0
    while lo < n:
        sz = sizes[i] if i < len(sizes) else rest
        i += 1
        hi = min(n, lo + sz)
        out.append((lo, hi))
        lo = hi
    return out


def _build_program(ne: int, pads: list[int]):
    nc = bacc.Bacc(
        "TRN2", target_bir_lowering=False, debug=False, num_devices=N_CORES
    )
    ntot = sum(pads)
    xoff = [0]
    for p_ in pads:
        xoff.append(xoff[-1] + p_)
    # x folded per unit: cols [2*xoff[u], +2*pads[u]) = [c0 block | c1 block],
    # so each batch is one contiguous per-partition run = 1 DMA descriptor
    # per partition (descriptor count, not bytes, is what DMA time costs)
    xt_d = nc.dram_tensor("xt", [P, 2 * ntot], MM_DT, kind="ExternalInput").ap()
    # flat weights with the (fp16) bias packed in the first 16 columns, so the
    # bias rides the head weight batch instead of its own 64-byte-descriptor
    # DMA: cols [0,16) = bias[p, u*2+ic], then 512 cols per unit (oc, ic, m)
    w_d = nc.dram_tensor(
        "wstat", [P, 2 * ne + ne * 4 * P], MM_DT, kind="ExternalInput"
    ).ap()
    y_d = nc.dram_tensor("y", [P, 2 * ntot], Y_DT, kind="ExternalOutput").ap()

    f32 = mybir.dt.float32
    relu = mybir.ActivationFunctionType.Relu
    add = mybir.AluOpType.add
    amax = mybir.AluOpType.max

    # batch plans; stagger so no unit's first matmul sees a new w-batch AND a
    # new x-batch at once (unit 0's two waits are absorbed by dummy matmuls)
    wbat = _batches(ne, [1, 4], ne)          # w: [0],[1..4],[5..7]
    xbat = _batches(ne, [4], ne)             # x: [0..3],[4..7]
    obat = _batches(ne, [2, 2, 2], 2)        # y out: per-pair, rotating rings

    u2w = {}
    for g, (lo, hi) in enumerate(wbat):
        for u in range(lo, hi):
            u2w[u] = g
    u2x = {}
    for bx, (lo, hi) in enumerate(xbat):
        for u in range(lo, hi):
            u2x[u] = bx

    with tile.TileContext(nc) as tc:
        with (
            tc.tile_pool(name="const", bufs=1) as const,
            tc.tile_pool(name="w", bufs=1) as wpool,
            tc.tile_pool(name="x", bufs=1) as xpool,
            tc.tile_pool(name="yout", bufs=1) as ypool,
            tc.tile_pool(name="ps", bufs=1, space="PSUM") as pspool,
            tc.tile_pool(name="scr", bufs=1, space="PSUM") as scrpool,
        ):
            xts = {}

            def load_x_batch(b, eng):
                lo, hi = xbat[b]
                span = xoff[hi] - xoff[lo]
                t = xpool.tile([P, 2 * span], MM_DT, tag=f"x{b}")
                xts[b] = t
                eng.dma_start(t[:], xt_d[:, 2 * xoff[lo]:2 * xoff[hi]])

            wts = {}

            def load_w_batch(g, eng):
                lo, hi = wbat[g]
                head = 2 * ne if g == 0 else 0
                t = wpool.tile(
                    [P, head + (hi - lo) * 4 * P], MM_DT, tag=f"w{g}"
                )
                wts[g] = t
                eng.dma_start(
                    t[:],
                    w_d[:, 2 * ne + lo * 4 * P - head:2 * ne + hi * 4 * P],
                )

            def xchunk(u, c):
                b = u2x[u]
                lo, _ = xbat[b]
                base = 2 * (xoff[u] - xoff[lo]) + c * pads[u]
                return xts[b][:, base:base + pads[u]]

            def wchunk(u, oc, ic):
                g = u2w[u]
                lo, _ = wbat[g]
                head = 2 * ne if g == 0 else 0
                base = head + ((u - lo) * 4 + oc * 2 + ic) * P
                return wts[g][:, base:base + P]

            # One DGE queue only sustains a fraction of the ~360 GB/s
            # HBM-per-core limit and small per-partition descriptors are
            # slower still, so the in-stream is spread over all three issue
            # rings (sync HWDGE, scalar HWDGE, gpsimd SWDGE) streaming
            # concurrently, in few large-descriptor batches.  Head batches
            # (w0+bias, x0) go first on their rings; same-ring FIFO delivery
            # keeps every later consumer to at most one new semaphore wait.
            # head-critical w0 and x0 ride DIFFERENT rings so both land at
            # the earliest possible time; second-wave batches queue behind,
            # one per ring, so no ring carries two large in-batches
            load_w_batch(0, nc.sync)     # bias + unit 0
            load_x_batch(0, nc.scalar)   # units 0-3
            load_w_batch(1, nc.gpsimd)   # units 1-4
            load_w_batch(2, nc.sync)     # units 5-7
            load_x_batch(1, nc.scalar)   # units 4-7
            bt = wts[0]                  # bias lives in w0's first 16 cols

            # ACT activation-table warm-up: pay the ~1.3us preamble while the
            # first DMA batches are still in flight.
            warm = const.tile([1, 2], f32, tag="warm")
            nc.vector.memset(warm[:], 0.0)
            nc.scalar.activation(warm[:], warm[:], relu)

            # PE p-state ramp: the tensor engine only reaches full clock
            # after ~3us of continuous work, so burn wide dummy matmuls on a
            # locally-initialized tile while the first DMA batches are still
            # in flight — the real stream then starts already warm.
            ramp = const.tile([P, 512], MM_DT, tag="ramp")
            nc.vector.memset(ramp[:], 0.0)

            # widen the packed fp16 bias to the fp32 per-partition scalars the
            # DVE/ACT drain ops require (one tiny ACT op once w0 lands)
            btf = const.tile([P, 2 * ne], f32, tag="btf")
            nc.scalar.copy(btf[:], bt[:, 0:2 * ne])

            scr = scrpool.tile([2, 512], f32)
            yt = ypool.tile([P, 2 * ntot], Y_DT, tag="y")

            for _ in range(9):
                nc.tensor.matmul(
                    scr[:], ramp[:, 0:2], ramp[:], start=True, stop=True
                )

            # Pack PSUM slots (one per (unit, ic)) multiple to a 2 KB bank;
            # allocated once up front so there are no pool-release waits.
            # Greedy first-fit of the per-unit widths into 512-f32 banks,
            # wrapping onto earlier banks (drained long before reuse) if the
            # 7 available banks run out.
            slot_loc = {}
            bank_fill = []
            for u in range(ne):
                for ic in range(2):
                    wdt = pads[u]
                    bi = next(
                        (i for i, f in enumerate(bank_fill) if f + wdt <= 512),
                        None,
                    )
                    if bi is None:
                        if len(bank_fill) < 7:
                            bank_fill.append(0)
                            bi = len(bank_fill) - 1
                        else:
                            bank_fill[:] = [0] * len(bank_fill)
                            bi = 0
                    slot_loc[u * 2 + ic] = (bi, bank_fill[bi])
                    bank_fill[bi] += wdt
            nbank = len(bank_fill)
            banks = [
                pspool.tile([P, 512], f32, name=f"pb{i}", tag=f"pb{i}")
                for i in range(nbank)
            ]

            def psum_slot(s, width):
                bi, off = slot_loc[s]
                return banks[bi][:, off:off + width]

            for u in range(ne):
                if u == 0:
                    # absorb the w-batch-0 and x-batch-0 semaphore waits so
                    # unit 0's real matmuls carry none
                    wb = wts[0]
                    nc.tensor.matmul(
                        scr[:, 0:2], wb[:, 0:2], wb[:, 0:2],
                        start=True, stop=True,
                    )
                    xb = xts[0]
                    nc.tensor.matmul(
                        scr[:, 0:2], xb[:, 0:2], xb[:, 0:2],
                        start=True, stop=True,
                    )
                for ic in range(2):
                    pd = pads[u]
                    ps = psum_slot(u * 2 + ic, pd)
                    nc.tensor.matmul(
                        ps, wchunk(u, 0, ic), xchunk(u, 0),
                        start=True, stop=False,
                    )
                    nc.tensor.matmul(
                        ps, wchunk(u, 1, ic), xchunk(u, 1),
                        start=False, stop=True,
                    )
                    o = 2 * xoff[u] + ic * pd
                    bv = btf[:, u * 2 + ic:u * 2 + ic + 1]
                    if ic == 0:
                        nc.vector.tensor_scalar(
                            yt[:, o:o + pd], ps, bv, 0.0, add, amax
                        )
                    else:
                        nc.scalar.activation(
                            yt[:, o:o + pd], ps, relu, bias=bv
                        )
                for oi, (lo, hi) in enumerate(obat):
                    if u == hi - 1:
                        # per-pair output batches on rotating rings, so the
                        # output stream overlaps the compute tail and the
                        # final transfer is small
                        eng = (nc.sync, nc.gpsimd, nc.scalar, nc.scalar)[oi]
                        eng.dma_start(
                            y_d[:, 2 * xoff[lo]:2 * xoff[hi]],
                            yt[:, 2 * xoff[lo]:2 * xoff[hi]],
                        )
    nc.compile()
    return nc


def kernel(x: np.ndarray, idxs: np.ndarray, w: np.ndarray, b: np.ndarray) -> np.ndarray:
    global LAST_EXEC_TIME_NS, LAST_TRACE
    x = np.ascontiguousarray(x, dtype=np.float32)
    w = np.ascontiguousarray(w, dtype=np.float32)
    b = np.ascontiguousarray(b, dtype=np.float32)
    idxs_np = np.asarray(idxs).astype(np.int64)

    B, Fdim = x.shape
    order, pads, ne, per_core = _build_units(idxs_np)
    ntot = sum(pads)
    xoff = np.concatenate([[0], np.cumsum(pads)]).astype(np.int64)

    # wprep[p, e, oc, ic, m] = w[e, ic*128+m, oc*128+p]
    wprep = np.ascontiguousarray(
        w.reshape(64, 2, P, 2, P).transpose(4, 0, 3, 1, 2).astype(NP_DT)
    )
    # bprep[p, e, ic] = b[e, ic*128+p]
    bprep = np.ascontiguousarray(b.reshape(64, 2, P).transpose(2, 0, 1))

    nc = _build_program(ne, pads)
    trace = bool(os.environ.get("KBENCH_TRACE"))

    in_maps = []
    for c in range(N_CORES):
        units = per_core[c]
        eids = np.array([u[0] for u in units])
        # xt[p, 2*xoff[u] + c*pads[u] + j] = x_sample(u,j)[c*128 + p]
        xt = np.zeros((P, 2 * ntot), dtype=NP_DT)
        for s, (e, k0, cnt) in enumerate(units):
            pd = pads[s]
            blk = np.zeros((pd, 2, P), dtype=np.float32)
            blk[:cnt] = x[order[k0:k0 + cnt]].reshape(cnt, 2, P)
            xt[:, 2 * xoff[s]:2 * xoff[s] + 2 * pd] = (
                blk.transpose(2, 1, 0).reshape(P, 2 * pd).astype(NP_DT)
            )
        wstat = np.ascontiguousarray(np.concatenate(
            [
                bprep[:, eids].reshape(P, 2 * ne).astype(NP_DT),
                wprep[:, eids].reshape(P, ne * 4 * P),
            ],
            axis=1,
        ))
        in_maps.append({"xt": xt, "wstat": wstat})

    res = run_bass_kernel_spmd(
        nc, in_maps, core_ids=list(range(N_CORES)), trace=trace
    )
    LAST_EXEC_TIME_NS = res.exec_time_ns
    LAST_TRACE = res.instructions_and_trace

    y = np.empty((B, Fdim), dtype=np.float32)
    for c in range(N_CORES):
        units = per_core[c]
        yc = res.results[c]["y"].astype(np.float32)  # [128, 2*ntot]
        for s, (e, k0, cnt) in enumerate(units):
            if cnt == 0:
                continue
            pd = pads[s]
            blk = yc[:, 2 * xoff[s]:2 * xoff[s] + 2 * pd]
            # blk[m, ic*pd + j] -> sample j, feature ic*128+m
            y[order[k0:k0 + cnt]] = (
                blk.reshape(P, 2, pd).transpose(2, 1, 0).reshape(pd, Fdim)[:cnt]
            )
    return y
